# revision 3
# baseline (speedup 1.0000x reference)
"""GAT layer (nn_GATLayer_28106265985525) on 8 Trainium2 NeuronCores.

Batch-parallel: core b computes graph b (bs=8). Math (per core):
  nodes = x @ W.T + b;  s[i,j] = fsrc_i + fdst_j + a_b;  p_i = fsrc_i + a_b
  attn  = softmax_j(adj ? lrelu(s) : -inf);  out = attn @ nodes

Key identity: exp(lrelu(s)) = max(e^s, e^{0.2 s}).  Dividing row i by
4*e^{0.2 p_i} (softmax-invariant) gives
  E[i,j] = adj * max(g_i * w_j, 1/4),  g = e^{0.8 p}/4, w = e^{0.8 fdst}
and the per-j factor v_j = e^{0.2 fdst_j} is folded into the GEMM
operand:  out = (E @ [v*nodes | v]) / Z  (Z from the v column).

Per 128-col block t (transposed layout, partitions = j):
  - 16 tensor_scalar ops (DVE 4x mode): T1[:,u] = max(Gb_t * w_u, 1/4)
  - one SWDGE DMA (per block pair) adds the host-prepared mask
    {adj=1: 0, adj=0: -120} (i8, pre-transposed) onto T1, casting inline
  - Relu zeroes masked entries exactly (split ACT / DVE / GPSIMD)
  - PE accumulates out_ps[i, 129] += E_u.T @ nE_u  (16 matmuls, K=128)
  - reciprocal of the Z column + ACT scale + DMA out (f16).
fsrc comes straight from x via q = W.T@a1 (no nodes dependency) and
fdst columns via 16 free K-contraction matmuls, so the setup critical
path is short.  No exp/lrelu/mask-multiply/PE-transpose in the loop.
"""

import dataclasses

import numpy as np
from contextlib import ExitStack

N = 2048
FIN = 256
F = 128
BS = 8
TB = N // 128  # 16 blocks
QUART = 0.25
LN4 = 1.3862943611198906

# relu column split: [0, ACT_C) on ACT, [ACT_C, ACT_C+DVE_C) on DVE, rest Pool
RELU_ACT = 256
RELU_DVE = 1792

_cache = {}


def _build(reps=1):
    import concourse.bass as bass
    import concourse.tile as tile
    from concourse import mybir, bacc

    f32, f16, i8 = mybir.dt.float32, mybir.dt.float16, mybir.dt.int8
    A = mybir.AluOpType
    AF = mybir.ActivationFunctionType

    nc = bacc.Bacc("TRN2", target_bir_lowering=False, debug=False)
    xt_d = nc.declare_dram_parameter("xT", [FIN, N], f16, isOutput=False)
    adjm_d = nc.declare_dram_parameter("adjm", [N, N], i8, isOutput=False)
    wt_d = nc.declare_dram_parameter("WwT", [FIN, F], f16, isOutput=False)
    ww_d = nc.declare_dram_parameter("Ww", [F, FIN], f16, isOutput=False)
    wb_d = nc.declare_dram_parameter("Wb", [F, 1], f32, isOutput=False)
    wb16_d = nc.declare_dram_parameter("Wb16", [F, 1], f16, isOutput=False)
    a12_d = nc.declare_dram_parameter("a12", [F, 2], f16, isOutput=False)
    ab_d = nc.declare_dram_parameter("ab", [1, 1], f32, isOutput=False)
    idf16_d = nc.declare_dram_parameter("idf16", [128, 128], f16, isOutput=False)
    out_d = nc.declare_dram_parameter("out", [N, F], f16, isOutput=True)

    with tile.TileContext(nc) as tc, ExitStack() as ctx:
        consts = ctx.enter_context(tc.tile_pool(name="consts", bufs=1))
        t1p = ctx.enter_context(tc.tile_pool(name="t1p", bufs=6))
        outp = ctx.enter_context(tc.tile_pool(name="outp", bufs=2))
        ps_nt = ctx.enter_context(tc.tile_pool(name="ps_nt", bufs=1, space="PSUM"))
        ps_gn = ctx.enter_context(tc.tile_pool(name="ps_gn", bufs=2, space="PSUM"))
        ps_sm = ctx.enter_context(tc.tile_pool(name="ps_sm", bufs=1, space="PSUM"))
        ps_out = ctx.enter_context(tc.tile_pool(name="ps_out", bufs=2, space="PSUM"))

        # -------- input DMAs: xT on SP, the rest on Activation HWDGE --------
        xt_sb = consts.tile([128, 2 * N], f16)
        nc.sync.dma_start(
            xt_sb[:].rearrange("p (c n) -> p c n", c=2),
            xt_d[:, :].rearrange("(c p) n -> p c n", p=128),
        )
        ww_sb = consts.tile([128, FIN], f16)
        nc.scalar.dma_start(ww_sb[:], ww_d[:, :])
        a12 = consts.tile([128, 2], f16)
        nc.scalar.dma_start(a12[:], a12_d[:, :])
        wb16 = consts.tile([128, 1], f16)
        nc.scalar.dma_start(wb16[:], wb16_d[:, :])
        ab_sb = consts.tile([1, 1], f32)
        nc.scalar.dma_start(ab_sb[:], ab_d[:, :])
        wt_sb = consts.tile([128, 2 * F], f16)
        nc.gpsimd.dma_start(
            wt_sb[:].rearrange("p (c o) -> p c o", c=2),
            wt_d[:, :].rearrange("(c p) o -> p c o", p=128),
        )
        wb_col = consts.tile([128, 1], f32)
        nc.gpsimd.dma_start(wb_col[:], wb_d[:, :])
        idf16 = consts.tile([128, 128], f16)
        nc.gpsimd.dma_start(idf16[:], idf16_d[:, :])

        # warm up the ACT engine off the critical path (first op pays a
        # large fixed cost in the cost model)
        warm = consts.tile([1, 8], f16)
        nc.vector.memset(warm[:], 0.0)
        nc.scalar.activation(warm[:], warm[:], AF.Copy)

        # -------- fsrc straight from x: q = W.T @ a1 (2 chunks), s_b = Wb.a1
        sm_ps = ps_sm.tile([128, 24], f32, tag="sm")
        q_ps = sm_ps[:, 0:4]
        for c in range(2):
            for j in range(2):
                nc.tensor.matmul(
                    sm_ps[:, 2 * j + c:2 * j + c + 1],
                    ww_sb[:, c * 128:(c + 1) * 128],
                    a12[:, j:j + 1], start=True, stop=True,
                )
        sb_ps = sm_ps[0:1, 4:5]
        nc.tensor.matmul(sb_ps, wb16[:], a12[:, 0:1], start=True, stop=True)
        sb2_ps = sm_ps[0:1, 5:6]
        nc.tensor.matmul(sb2_ps, wb16[:], a12[:, 1:2], start=True, stop=True)
        ones_row = consts.tile([1, 128], f16)
        nc.vector.memset(ones_row[:], 1.0)
        q16 = consts.tile([128, 4], f16)
        nc.scalar.activation(q16[:], q_ps, AF.Copy)
        sb2_16 = consts.tile([1, 1], f16)
        nc.scalar.activation(sb2_16[:], sb2_ps, AF.Copy)
        # g bias: 0.8*(ab + Wb.a1) - ln4
        b_g = consts.tile([1, 1], f32)
        nc.vector.tensor_scalar(b_g[:], sb_ps, ab_sb[:], 0.8, A.add, A.mult)
        nc.vector.tensor_scalar(b_g[:], b_g[:], -LN4, None, A.add)
        # fsrc row [1, 2048] (no bias folded; it lives in b_g)
        fs_ps = ps_big.tile([1, N], f32, tag="big")
        for ch in range(4):
            for c in range(2):
                nc.tensor.matmul(
                    fs_ps[:, ch * 512:(ch + 1) * 512],
                    q16[:, c:c + 1],
                    xt_sb[:, c * N + ch * 512: c * N + ch * 512 + 512],
                    start=(c == 0), stop=(c == 1),
                )
        g_row = consts.tile([1, N], f16)
        nc.scalar.activation(g_row[:], fs_ps[:], AF.Exp, bias=b_g[:], scale=0.8)

        # -------- fdst columns straight from x (32 tiny matmuls) + wb.a2 row
        fd_ps = sm_ps[:, 8:24]
        for u in range(TB):
            for c in range(2):
                nc.tensor.matmul(
                    fd_ps[:, u:u + 1],
                    xt_sb[:, c * N + u * 128: c * N + u * 128 + 128],
                    q16[:, 2 + c:3 + c], start=(c == 0), stop=False,
                )
            nc.tensor.matmul(
                fd_ps[:, u:u + 1], ones_row[:], sb2_16[:],
                start=False, stop=True,
            )

        # -------- Gb[p, i] = g_i broadcast (outer product, chunked evac)
        ones_row = consts.tile([1, 128], f16)
        nc.vector.memset(ones_row[:], 1.0)
        gb_ps = ps_big.tile([128, N], f32, tag="big")
        for ch in range(4):
            fs_ps = ps_gn.tile([1, 512], f32, tag="gn", name="fs_ps")
            for c in range(2):
                nc.tensor.matmul(
                    fs_ps[:],
                    q16[:, c:c + 1],
                    xt_sb[:, c * N + ch * 512: c * N + ch * 512 + 512],
                    start=(c == 0), stop=(c == 1),
                )
            nc.scalar.activation(
                g_row[:, ch * 512:(ch + 1) * 512], fs_ps[:],
                AF.Exp, bias=b_g[:], scale=0.8,
            )
        # -------- Gb[p, i] = g_i broadcast, chunked outer + evac pipeline
        gb = consts.tile([128, N], f16)
        for ch in range(4):
            gb_ps = ps_gn.tile([128, 512], f32, tag="gn", name="gb_ps")
            nc.tensor.matmul(
                gb_ps[:], ones_row[:],
                g_row[:, ch * 512:(ch + 1) * 512], start=True, stop=True,
            )
            nc.scalar.activation(
                gb[:, ch * 512:(ch + 1) * 512], gb_ps[:], AF.Copy,
            )

        # -------- nodes^T = W @ x^T + b -> f16
        nT_ps = ps_big.tile([128, N], f32, tag="big")
        for nch in range(4):
            for c in range(2):
                nc.tensor.matmul(
                    nT_ps[:, nch * 512:(nch + 1) * 512],
                    wt_sb[:, c * F:(c + 1) * F],
                    xt_sb[:, c * N + nch * 512: c * N + nch * 512 + 512],
                    start=(c == 0), stop=(c == 1),
                )
        nT16 = consts.tile([128, N], f16)
        nc.scalar.activation(nT16[:], nT_ps[:], AF.Identity, bias=wb_col[:])

        bln4 = consts.tile([128, 1], f32)
        nc.vector.memset(bln4[:], -LN4)
        c_cols = consts.tile([128, TB], f32)
        # -------- Gb[p, i] = g_i broadcast, chunk 0 first (gates block 0)
        gb = consts.tile([128, N], f16)
        gb_ps0 = ps_gn.tile([128, 512], f32, tag="gn", name="gb_ps0")
        nc.tensor.matmul(gb_ps0[:], ones_row[:], g_row[:, 0:512],
                         start=True, stop=True)
        nc.scalar.activation(gb[:, 0:512], gb_ps0[:], AF.Copy)
        nc.scalar.activation(c_cols[:], fd_ps, AF.Exp, scale=-0.8, bias=bln4[:])
        wv_cols = consts.tile([128, TB], f32)
        nc.scalar.activation(wv_cols[:], fd_ps, AF.Exp, scale=1.0)
        # Cbig[p, (u, i)] = c_{u*128+p}
        cbig = consts.tile([128, N], f16)
        ones128 = consts.tile([128, 128], f16)
        nc.vector.memset(ones128[:], 1.0)
        for u in range(TB):
            nc.vector.tensor_scalar(
                cbig[:, u * 128:(u + 1) * 128], ones128[:],
                c_cols[:, u:u + 1], None, A.mult,
            )

        # -------- nE[p, (u, e)] f16: cols 0..127 = v_j*nodes[j,:], col 128 = v_j
        nE_sb = consts.tile([128, TB * 129], f16)
        nE_v = nE_sb[:].rearrange("p (t e) -> p t e", e=129)
        for g4 in range(4):
            nE_ps = ps_gn.tile([128, 512], f16, tag="gn", name="nE_ps")
            for k in range(4):
                t = g4 * 4 + k
                nc.tensor.transpose(
                    nE_ps[:, k * 128:(k + 1) * 128],
                    nT16[:, t * 128:(t + 1) * 128],
                    idf16[:],
                )
            for k in range(4):
                u = g4 * 4 + k
                nc.scalar.activation(
                    nE_v[:, u, 0:128], nE_ps[:, k * 128:(k + 1) * 128],
                    AF.Copy, scale=v_cols[:, u:u + 1],
                )
        nc.vector.tensor_copy(
            nE_v[:, :, 128:129], wv_cols[:].rearrange("p (t o) -> p t o", o=1)
        )

        # -------- main loop: block pairs share one mask DMA --------
        RA, RD = RELU_ACT, RELU_DVE
        for tp in [tp for _ in range(reps) for tp in range(TB // 2)]:
            t1d = t1p.tile([128, 2 * N], f16, tag="t1")
            for h in range(2):
                t = 2 * tp + h
                for u in range(TB):
                    nc.vector.tensor_scalar(
                        t1d[:, h * N + u * 128: h * N + (u + 1) * 128],
                        gb[:, t * 128:(t + 1) * 128],
                        w_cols[:, u:u + 1], QUART, A.mult, A.max,
                    )
            nc.gpsimd.dma_start(
                t1d[:].rearrange("p (h m) -> p h m", h=2),
                adjm_d[2 * tp * 128:(2 * tp + 2) * 128, :].rearrange(
                    "(h p) m -> p h m", p=128),
                accum_op=A.add,
            )
            for h in range(2):
                t = 2 * tp + h
                o = h * N
                et = etp.tile([128, N], f16, tag="et")
                nc.scalar.activation(
                    et[:, 0:RA], t1d[:, o:o + RA], AF.Relu)
                nc.vector.tensor_scalar(
                    et[:, RA:RA + RD], t1d[:, o + RA:o + RA + RD],
                    0.0, None, A.max)
                nc.gpsimd.tensor_scalar(
                    et[:, RA + RD:], t1d[:, o + RA + RD:o + N],
                    0.0, None, A.max)

                out_ps = ps_out.tile([128, 129], f32, tag="out")
                for u in range(TB):
                    nc.tensor.matmul(
                        out_ps[:],
                        et[:, u * 128:(u + 1) * 128],
                        nE_v[:, u, :],
                        start=(u == 0), stop=(u == TB - 1),
                    )
                rcp = outp.tile([128, 1], f32, tag="rcp")
                nc.vector.reciprocal(rcp[:], out_ps[:, 128:129])
                osb = outp.tile([128, F], f16, tag="osb")
                nc.scalar.activation(osb[:], out_ps[:, 0:F], AF.Copy,
                                     scale=rcp[:])
                nc.sync.dma_start(out_d[t * 128:(t + 1) * 128, :], osb[:])

    nc.compile()
    return nc


def make_in_maps(inputs, adjs, W_w, W_b, a_w, a_b):
    xT = np.ascontiguousarray(
        np.asarray(inputs, dtype=np.float32).transpose(0, 2, 1)
    ).astype(np.float16)
    a = np.asarray(adjs, dtype=np.int8)
    # adjm[t*128+p, u*128+i] = -120*(1 - adj[b, t*128+i, u*128+p])
    a4 = a.reshape(BS, TB, 128, TB, 128)  # [b, t, i, u, p]
    adjm = ((a4.transpose(0, 1, 4, 3, 2).astype(np.int16) - 1) * 120).astype(
        np.int8
    ).reshape(BS, N, N)
    ww = np.ascontiguousarray(np.asarray(W_w, dtype=np.float32)).astype(np.float16)
    wwT = np.ascontiguousarray(ww.T)
    wb = np.ascontiguousarray(W_b, dtype=np.float32).reshape(F, 1)
    aw = np.asarray(a_w, dtype=np.float32)
    a12 = np.stack([aw[0, :F], aw[0, F:]], axis=1).astype(np.float16)
    ab = np.asarray(a_b, dtype=np.float32).reshape(1, 1).copy()
    idf16 = np.eye(128, dtype=np.float16)
    return [
        {
            "xT": xT[b],
            "adjm": adjm[b],
            "WwT": wwT,
            "Ww": ww,
            "Wb": wb,
            "Wb16": wb.astype(np.float16),
            "a12": a12,
            "ab": ab,
            "idf16": idf16,
        }
        for b in range(BS)
    ]


def kernel(inputs, adjs, W_w, W_b, a_w, a_b):
    from concourse.bass_utils import run_bass_kernel_spmd

    if "nc" not in _cache:
        _cache["nc"] = _build()
    nc = _cache["nc"]

    in_maps = make_in_maps(inputs, adjs, W_w, W_b, a_w, a_b)
    try:
        res = run_bass_kernel_spmd(nc, in_maps, core_ids=list(range(BS)))
    except Exception:
        res = run_bass_kernel_spmd(nc, in_maps, core_ids=list(range(BS)))
    out = np.stack([res.results[b]["out"] for b in range(BS)], axis=0)
    return out.astype(np.float32)


# revision 4
# speedup vs baseline: 1.0237x; 1.0237x over previous
"""GAT layer (nn_GATLayer_28106265985525) on 8 Trainium2 NeuronCores.

Batch-parallel: core b computes graph b (bs=8). Math (per core):
  nodes = x @ W.T + b;  s[i,j] = fsrc_i + fdst_j + a_b;  p_i = fsrc_i + a_b
  attn  = softmax_j(adj ? lrelu(s) : -inf);  out = attn @ nodes

Key identity: exp(lrelu(s)) = max(e^s, e^{0.2 s}).  Dividing row i by
4*e^{0.2 p_i} (softmax-invariant) gives
  E[i,j] = adj * max(g_i * w_j, 1/4),  g = e^{0.8 p}/4, w = e^{0.8 fdst}
and the per-j factor v_j = e^{0.2 fdst_j} is folded into the GEMM
operand:  out = (E @ [v*nodes | v]) / Z  (Z from the v column).

Per 128-col block t (transposed layout, partitions = j):
  - 16 tensor_scalar ops (DVE 4x mode): T1[:,u] = max(Gb_t * w_u, 1/4)
  - one SWDGE DMA (per block pair) adds the host-prepared mask
    {adj=1: 0, adj=0: -120} (i8, pre-transposed) onto T1, casting inline
  - Relu zeroes masked entries exactly (split ACT / DVE / GPSIMD)
  - PE accumulates out_ps[i, 129] += E_u.T @ nE_u  (16 matmuls, K=128)
  - reciprocal of the Z column + ACT scale + DMA out (f16).
fsrc comes straight from x via q = W.T@a1 (no nodes dependency) and
fdst columns via 16 free K-contraction matmuls, so the setup critical
path is short.  No exp/lrelu/mask-multiply/PE-transpose in the loop.
"""

import dataclasses

import numpy as np
from contextlib import ExitStack

N = 2048
FIN = 256
F = 128
BS = 8
TB = N // 128  # 16 blocks
QUART = 0.25
LN4 = 1.3862943611198906

# relu column split: [0, ACT_C) on ACT, [ACT_C, ACT_C+DVE_C) on DVE, rest Pool
RELU_ACT = 320
RELU_DVE = 1728

_cache = {}


def _build(reps=1):
    import concourse.bass as bass
    import concourse.tile as tile
    from concourse import mybir, bacc

    f32, f16, i8 = mybir.dt.float32, mybir.dt.float16, mybir.dt.int8
    A = mybir.AluOpType
    AF = mybir.ActivationFunctionType

    nc = bacc.Bacc("TRN2", target_bir_lowering=False, debug=False)
    xt_d = nc.declare_dram_parameter("xT", [FIN, N], f16, isOutput=False)
    adjm_d = nc.declare_dram_parameter("adjm", [N, N], i8, isOutput=False)
    wt_d = nc.declare_dram_parameter("WwT", [FIN, F], f16, isOutput=False)
    ww_d = nc.declare_dram_parameter("Ww", [F, FIN], f16, isOutput=False)
    wb_d = nc.declare_dram_parameter("Wb", [F, 1], f32, isOutput=False)
    wb16_d = nc.declare_dram_parameter("Wb16", [F, 1], f16, isOutput=False)
    a12_d = nc.declare_dram_parameter("a12", [F, 2], f16, isOutput=False)
    ab_d = nc.declare_dram_parameter("ab", [1, 1], f32, isOutput=False)
    idf16_d = nc.declare_dram_parameter("idf16", [128, 128], f16, isOutput=False)
    out_d = nc.declare_dram_parameter("out", [N, F], f16, isOutput=True)

    with tile.TileContext(nc) as tc, ExitStack() as ctx:
        consts = ctx.enter_context(tc.tile_pool(name="consts", bufs=1))
        t1p = ctx.enter_context(tc.tile_pool(name="t1p", bufs=7))
        outp = ctx.enter_context(tc.tile_pool(name="outp", bufs=2))
        ps_nt = ctx.enter_context(tc.tile_pool(name="ps_nt", bufs=1, space="PSUM"))
        ps_gn = ctx.enter_context(tc.tile_pool(name="ps_gn", bufs=2, space="PSUM"))
        ps_sm = ctx.enter_context(tc.tile_pool(name="ps_sm", bufs=1, space="PSUM"))
        ps_out = ctx.enter_context(tc.tile_pool(name="ps_out", bufs=2, space="PSUM"))

        # -------- input DMAs: xT on SP, the rest on Activation HWDGE --------
        xt_sb = consts.tile([128, 2 * N], f16)
        nc.sync.dma_start(
            xt_sb[:].rearrange("p (c n) -> p c n", c=2),
            xt_d[:, :].rearrange("(c p) n -> p c n", p=128),
        )
        ww_sb = consts.tile([128, FIN], f16)
        nc.scalar.dma_start(ww_sb[:], ww_d[:, :])
        a12 = consts.tile([128, 2], f16)
        nc.scalar.dma_start(a12[:], a12_d[:, :])
        wb16 = consts.tile([128, 1], f16)
        nc.scalar.dma_start(wb16[:], wb16_d[:, :])
        ab_sb = consts.tile([1, 1], f32)
        nc.scalar.dma_start(ab_sb[:], ab_d[:, :])
        wt_sb = consts.tile([128, 2 * F], f16)
        nc.gpsimd.dma_start(
            wt_sb[:].rearrange("p (c o) -> p c o", c=2),
            wt_d[:, :].rearrange("(c p) o -> p c o", p=128),
        )
        wb_col = consts.tile([128, 1], f32)
        nc.gpsimd.dma_start(wb_col[:], wb_d[:, :])
        idf16 = consts.tile([128, 128], f16)
        nc.gpsimd.dma_start(idf16[:], idf16_d[:, :])

        # warm up the ACT engine off the critical path (first op pays a
        # large fixed cost in the cost model)
        warm = consts.tile([1, 8], f16)
        nc.vector.memset(warm[:], 0.0)
        nc.scalar.activation(warm[:], warm[:], AF.Copy)

        # -------- fsrc straight from x: q = W.T @ a1 (2 chunks), s_b = Wb.a1
        sm_ps = ps_sm.tile([128, 24], f32, tag="sm")
        q_ps = sm_ps[:, 0:4]
        for c in range(2):
            for j in range(2):
                nc.tensor.matmul(
                    sm_ps[:, 2 * j + c:2 * j + c + 1],
                    ww_sb[:, c * 128:(c + 1) * 128],
                    a12[:, j:j + 1], start=True, stop=True,
                )
        sb_ps = sm_ps[0:1, 4:5]
        nc.tensor.matmul(sb_ps, wb16[:], a12[:, 0:1], start=True, stop=True)
        sb2_ps = sm_ps[0:1, 5:6]
        nc.tensor.matmul(sb2_ps, wb16[:], a12[:, 1:2], start=True, stop=True)
        ones_row = consts.tile([1, 128], f16)
        nc.vector.memset(ones_row[:], 1.0)
        q16 = consts.tile([128, 4], f16)
        nc.scalar.activation(q16[:], q_ps, AF.Copy)
        sb2_16 = consts.tile([1, 1], f16)
        nc.scalar.activation(sb2_16[:], sb2_ps, AF.Copy)
        # g bias: 0.8*(ab + Wb.a1) - ln4
        b_g = consts.tile([1, 1], f32)
        nc.vector.tensor_scalar(b_g[:], sb_ps, ab_sb[:], 0.8, A.add, A.mult)
        nc.vector.tensor_scalar(b_g[:], b_g[:], -LN4, None, A.add)
        # fsrc row [1, 2048] (no bias folded; it lives in b_g)
        fs_ps = ps_big.tile([1, N], f32, tag="big")
        for ch in range(4):
            for c in range(2):
                nc.tensor.matmul(
                    fs_ps[:, ch * 512:(ch + 1) * 512],
                    q16[:, c:c + 1],
                    xt_sb[:, c * N + ch * 512: c * N + ch * 512 + 512],
                    start=(c == 0), stop=(c == 1),
                )
        g_row = consts.tile([1, N], f16)
        nc.scalar.activation(g_row[:], fs_ps[:], AF.Exp, bias=b_g[:], scale=0.8)

        # -------- fdst columns straight from x (32 tiny matmuls) + wb.a2 row
        fd_ps = sm_ps[:, 8:24]
        for u in range(TB):
            for c in range(2):
                nc.tensor.matmul(
                    fd_ps[:, u:u + 1],
                    xt_sb[:, c * N + u * 128: c * N + u * 128 + 128],
                    q16[:, 2 + c:3 + c], start=(c == 0), stop=False,
                )
            nc.tensor.matmul(
                fd_ps[:, u:u + 1], ones_row[:], sb2_16[:],
                start=False, stop=True,
            )

        # -------- Gb[p, i] = g_i broadcast (outer product, chunked evac)
        ones_row = consts.tile([1, 128], f16)
        nc.vector.memset(ones_row[:], 1.0)
        gb_ps = ps_big.tile([128, N], f32, tag="big")
        gb = consts.tile([128, N], f16)

        def fs_chunk(ch):
            fs_ps = ps_gn.tile([1, 512], f32, tag="gn", name="fs_ps")
            for c in range(2):
                nc.tensor.matmul(
                    fs_ps[:],
                    q16[:, c:c + 1],
                    xt_sb[:, c * N + ch * 512: c * N + ch * 512 + 512],
                    start=(c == 0), stop=(c == 1),
                )
            nc.scalar.activation(
                g_row[:, ch * 512:(ch + 1) * 512], fs_ps[:],
                AF.Exp, bias=b_g[:], scale=0.8,
            )

        def gb_chunk(ch):
            gb_ps = ps_gn.tile([128, 512], f32, tag="gn", name="gb_ps")
            nc.tensor.matmul(
                gb_ps[:], ones_row[:],
                g_row[:, ch * 512:(ch + 1) * 512], start=True, stop=True,
            )
            nc.scalar.activation(
                gb[:, ch * 512:(ch + 1) * 512], gb_ps[:], AF.Copy,
            )

        fs_chunk(0)
        fs_chunk(1)
        gb_chunk(0)
        fs_chunk(2)
        gb_chunk(1)
        fs_chunk(3)
        gb_chunk(2)
        gb_chunk(3)

        # -------- nodes^T = W @ x^T + b -> f16
        nT_ps = ps_big.tile([128, N], f32, tag="big")
        for nch in range(4):
            for c in range(2):
                nc.tensor.matmul(
                    nT_ps[:, nch * 512:(nch + 1) * 512],
                    wt_sb[:, c * F:(c + 1) * F],
                    xt_sb[:, c * N + nch * 512: c * N + nch * 512 + 512],
                    start=(c == 0), stop=(c == 1),
                )
        nT16 = consts.tile([128, N], f16)
        nc.scalar.activation(nT16[:], nT_ps[:], AF.Identity, bias=wb_col[:])

        bln4 = consts.tile([128, 1], f32)
        nc.vector.memset(bln4[:], -LN4)
        c_cols = consts.tile([128, TB], f32)
        # -------- Gb[p, i] = g_i broadcast, chunk 0 first (gates block 0)
        gb = consts.tile([128, N], f16)
        gb_ps0 = ps_gn.tile([128, 512], f32, tag="gn", name="gb_ps0")
        nc.tensor.matmul(gb_ps0[:], ones_row[:], g_row[:, 0:512],
                         start=True, stop=True)
        nc.scalar.activation(gb[:, 0:512], gb_ps0[:], AF.Copy)
        nc.scalar.activation(c_cols[:], fd_ps, AF.Exp, scale=-0.8, bias=bln4[:])
        wv_cols = consts.tile([128, TB], f32)
        nc.scalar.activation(wv_cols[:], fd_ps, AF.Exp, scale=1.0)
        # Cbig[p, (u, i)] = c_{u*128+p}
        cbig = consts.tile([128, N], f16)
        ones128 = consts.tile([128, 128], f16)
        nc.vector.memset(ones128[:], 1.0)
        for u in range(TB):
            nc.vector.tensor_scalar(
                cbig[:, u * 128:(u + 1) * 128], ones128[:],
                c_cols[:, u:u + 1], None, A.mult,
            )

        # -------- nE[p, (u, e)] f16: cols 0..127 = v_j*nodes[j,:], col 128 = v_j
        nE_sb = consts.tile([128, TB * 129], f16)
        nE_v = nE_sb[:].rearrange("p (t e) -> p t e", e=129)
        for g4 in range(4):
            nE_ps = ps_gn.tile([128, 512], f16, tag="gn", name="nE_ps")
            for k in range(4):
                t = g4 * 4 + k
                nc.tensor.transpose(
                    nE_ps[:, k * 128:(k + 1) * 128],
                    nT16[:, t * 128:(t + 1) * 128],
                    idf16[:],
                )
            for k in range(4):
                u = g4 * 4 + k
                nc.scalar.activation(
                    nE_v[:, u, 0:128], nE_ps[:, k * 128:(k + 1) * 128],
                    AF.Copy, scale=v_cols[:, u:u + 1],
                )
        nc.vector.tensor_copy(
            nE_v[:, :, 128:129], wv_cols[:].rearrange("p (t o) -> p t o", o=1)
        )

        # -------- main loop: block pairs share one mask DMA --------
        RA, RD = RELU_ACT, RELU_DVE
        for tp in [tp for _ in range(reps) for tp in range(TB // 2)]:
            t1d = t1p.tile([128, 2 * N], f16, tag="t1")
            for h in range(2):
                t = 2 * tp + h
                for u in range(TB):
                    nc.vector.tensor_scalar(
                        t1d[:, h * N + u * 128: h * N + (u + 1) * 128],
                        gb[:, t * 128:(t + 1) * 128],
                        w_cols[:, u:u + 1], QUART, A.mult, A.max,
                    )
            nc.gpsimd.dma_start(
                t1d[:].rearrange("p (h m) -> p h m", h=2),
                adjm_d[2 * tp * 128:(2 * tp + 2) * 128, :].rearrange(
                    "(h p) m -> p h m", p=128),
                accum_op=A.add,
            )
            for h in range(2):
                t = 2 * tp + h
                o = h * N
                et = etp.tile([128, N], f16, tag="et")
                nc.scalar.activation(
                    et[:, 0:RA], t1d[:, o:o + RA], AF.Relu)
                nc.vector.tensor_scalar(
                    et[:, RA:RA + RD], t1d[:, o + RA:o + RA + RD],
                    0.0, None, A.max)
                nc.gpsimd.tensor_scalar(
                    et[:, RA + RD:], t1d[:, o + RA + RD:o + N],
                    0.0, None, A.max)

                out_ps = ps_out.tile([128, 129], f32, tag="out")
                for u in range(TB):
                    nc.tensor.matmul(
                        out_ps[:],
                        et[:, u * 128:(u + 1) * 128],
                        nE_v[:, u, :],
                        start=(u == 0), stop=(u == TB - 1),
                    )
                rcp = outp.tile([128, 1], f32, tag="rcp")
                nc.vector.reciprocal(rcp[:], out_ps[:, 128:129])
                osb = outp.tile([128, F], f16, tag="osb")
                nc.scalar.activation(osb[:], out_ps[:, 0:F], AF.Copy,
                                     scale=rcp[:])
                nc.sync.dma_start(out_d[t * 128:(t + 1) * 128, :], osb[:])

    nc.compile()
    return nc


def make_in_maps(inputs, adjs, W_w, W_b, a_w, a_b):
    xT = np.ascontiguousarray(
        np.asarray(inputs, dtype=np.float32).transpose(0, 2, 1)
    ).astype(np.float16)
    a = np.asarray(adjs, dtype=np.int8)
    # adjm[t*128+p, u*128+i] = -120*(1 - adj[b, t*128+i, u*128+p])
    a4 = a.reshape(BS, TB, 128, TB, 128)  # [b, t, i, u, p]
    adjm = ((a4.transpose(0, 1, 4, 3, 2).astype(np.int16) - 1) * 120).astype(
        np.int8
    ).reshape(BS, N, N)
    ww = np.ascontiguousarray(np.asarray(W_w, dtype=np.float32)).astype(np.float16)
    wwT = np.ascontiguousarray(ww.T)
    wb = np.ascontiguousarray(W_b, dtype=np.float32).reshape(F, 1)
    aw = np.asarray(a_w, dtype=np.float32)
    a12 = np.stack([aw[0, :F], aw[0, F:]], axis=1).astype(np.float16)
    ab = np.asarray(a_b, dtype=np.float32).reshape(1, 1).copy()
    idf16 = np.eye(128, dtype=np.float16)
    return [
        {
            "xT": xT[b],
            "adjm": adjm[b],
            "WwT": wwT,
            "Ww": ww,
            "Wb": wb,
            "Wb16": wb.astype(np.float16),
            "a12": a12,
            "ab": ab,
            "idf16": idf16,
        }
        for b in range(BS)
    ]


def kernel(inputs, adjs, W_w, W_b, a_w, a_b):
    from concourse.bass_utils import run_bass_kernel_spmd

    if "nc" not in _cache:
        _cache["nc"] = _build()
    nc = _cache["nc"]

    in_maps = make_in_maps(inputs, adjs, W_w, W_b, a_w, a_b)
    try:
        res = run_bass_kernel_spmd(nc, in_maps, core_ids=list(range(BS)))
    except Exception:
        res = run_bass_kernel_spmd(nc, in_maps, core_ids=list(range(BS)))
    out = np.stack([res.results[b]["out"] for b in range(BS)], axis=0)
    return out.astype(np.float32)


# revision 5
# speedup vs baseline: 1.0239x; 1.0002x over previous
"""GAT layer (nn_GATLayer_28106265985525) on 8 Trainium2 NeuronCores.

Batch-parallel: core b computes graph b (bs=8). Math (per core):
  nodes = x @ W.T + b;  s[i,j] = fsrc_i + fdst_j + a_b;  p_i = fsrc_i + a_b
  attn  = softmax_j(adj ? lrelu(s) : -inf);  out = attn @ nodes

Key identity: exp(lrelu(s)) = max(e^s, e^{0.2 s}).  Dividing row i by
4*e^{0.2 p_i} (softmax-invariant) gives
  E[i,j] = adj * max(g_i * w_j, 1/4),  g = e^{0.8 p}/4, w = e^{0.8 fdst}
and the per-j factor v_j = e^{0.2 fdst_j} is folded into the GEMM
operand:  out = (E @ [v*nodes | v]) / Z  (Z from the v column).

Per 128-col block t (transposed layout, partitions = j):
  - 16 tensor_scalar ops (DVE 4x mode): T1[:,u] = max(Gb_t * w_u, 1/4)
  - one SWDGE DMA (per block pair) adds the host-prepared mask
    {adj=1: 0, adj=0: -120} (i8, pre-transposed) onto T1, casting inline
  - Relu zeroes masked entries exactly (split ACT / DVE / GPSIMD)
  - PE accumulates out_ps[i, 129] += E_u.T @ nE_u  (16 matmuls, K=128)
  - reciprocal of the Z column + ACT scale + DMA out (f16).
fsrc comes straight from x via q = W.T@a1 (no nodes dependency) and
fdst columns via 16 free K-contraction matmuls, so the setup critical
path is short.  No exp/lrelu/mask-multiply/PE-transpose in the loop.
"""

import dataclasses

import numpy as np
from contextlib import ExitStack

N = 2048
FIN = 256
F = 128
BS = 8
TB = N // 128  # 16 blocks
QUART = 0.25
LN4 = 1.3862943611198906

# relu column split: [0, ACT_C) on ACT, [ACT_C, ACT_C+DVE_C) on DVE, rest Pool
RELU_ACT = 320
RELU_DVE = 1728

_cache = {}


def _build(reps=1):
    import concourse.bass as bass
    import concourse.tile as tile
    from concourse import mybir, bacc

    f32, f16, i8 = mybir.dt.float32, mybir.dt.float16, mybir.dt.int8
    A = mybir.AluOpType
    AF = mybir.ActivationFunctionType

    nc = bacc.Bacc("TRN2", target_bir_lowering=False, debug=False)
    xt_d = nc.declare_dram_parameter("xT", [FIN, N], f16, isOutput=False)
    adjm_d = nc.declare_dram_parameter("adjm", [N, N], i8, isOutput=False)
    wt_d = nc.declare_dram_parameter("WwT", [FIN, F], f16, isOutput=False)
    ww_d = nc.declare_dram_parameter("Ww", [F, FIN], f16, isOutput=False)
    wb_d = nc.declare_dram_parameter("Wb", [F, 1], f32, isOutput=False)
    wb16_d = nc.declare_dram_parameter("Wb16", [F, 1], f16, isOutput=False)
    a12_d = nc.declare_dram_parameter("a12", [F, 2], f16, isOutput=False)
    ab_d = nc.declare_dram_parameter("ab", [1, 1], f32, isOutput=False)
    idf16_d = nc.declare_dram_parameter("idf16", [128, 128], f16, isOutput=False)
    out_d = nc.declare_dram_parameter("out", [N, F], f16, isOutput=True)

    with tile.TileContext(nc) as tc, ExitStack() as ctx:
        consts = ctx.enter_context(tc.tile_pool(name="consts", bufs=1))
        t1p = ctx.enter_context(tc.tile_pool(name="t1p", bufs=7))
        outp = ctx.enter_context(tc.tile_pool(name="outp", bufs=3))
        ps_nt = ctx.enter_context(tc.tile_pool(name="ps_nt", bufs=1, space="PSUM"))
        ps_gn = ctx.enter_context(tc.tile_pool(name="ps_gn", bufs=2, space="PSUM"))
        ps_sm = ctx.enter_context(tc.tile_pool(name="ps_sm", bufs=1, space="PSUM"))
        ps_out = ctx.enter_context(tc.tile_pool(name="ps_out", bufs=3, space="PSUM"))

        # -------- input DMAs: xT on SP, the rest on Activation HWDGE --------
        xt_sb = consts.tile([128, 2 * N], f16)
        nc.sync.dma_start(
            xt_sb[:].rearrange("p (c n) -> p c n", c=2),
            xt_d[:, :].rearrange("(c p) n -> p c n", p=128),
        )
        ww_sb = consts.tile([128, FIN], f16)
        nc.scalar.dma_start(ww_sb[:], ww_d[:, :])
        a12 = consts.tile([128, 2], f16)
        nc.scalar.dma_start(a12[:], a12_d[:, :])
        wb16 = consts.tile([128, 1], f16)
        nc.scalar.dma_start(wb16[:], wb16_d[:, :])
        ab_sb = consts.tile([1, 1], f32)
        nc.scalar.dma_start(ab_sb[:], ab_d[:, :])
        wt_sb = consts.tile([128, 2 * F], f16)
        nc.gpsimd.dma_start(
            wt_sb[:].rearrange("p (c o) -> p c o", c=2),
            wt_d[:, :].rearrange("(c p) o -> p c o", p=128),
        )
        wb_col = consts.tile([128, 1], f32)
        nc.gpsimd.dma_start(wb_col[:], wb_d[:, :])
        idf16 = consts.tile([128, 128], f16)
        nc.gpsimd.dma_start(idf16[:], idf16_d[:, :])

        # warm up the ACT engine off the critical path (first op pays a
        # large fixed cost in the cost model)
        warm = consts.tile([1, 8], f16)
        nc.vector.memset(warm[:], 0.0)
        nc.scalar.activation(warm[:], warm[:], AF.Copy)

        # -------- fsrc straight from x: q = W.T @ a1 (2 chunks), s_b = Wb.a1
        sm_ps = ps_sm.tile([128, 24], f32, tag="sm")
        q_ps = sm_ps[:, 0:4]
        for c in range(2):
            for j in range(2):
                nc.tensor.matmul(
                    sm_ps[:, 2 * j + c:2 * j + c + 1],
                    ww_sb[:, c * 128:(c + 1) * 128],
                    a12[:, j:j + 1], start=True, stop=True,
                )
        sb_ps = sm_ps[0:1, 4:5]
        nc.tensor.matmul(sb_ps, wb16[:], a12[:, 0:1], start=True, stop=True)
        sb2_ps = sm_ps[0:1, 5:6]
        nc.tensor.matmul(sb2_ps, wb16[:], a12[:, 1:2], start=True, stop=True)
        ones_row = consts.tile([1, 128], f16)
        nc.vector.memset(ones_row[:], 1.0)
        q16 = consts.tile([128, 4], f16)
        nc.scalar.activation(q16[:], q_ps, AF.Copy)
        sb2_16 = consts.tile([1, 1], f16)
        nc.scalar.activation(sb2_16[:], sb2_ps, AF.Copy)
        # g bias: 0.8*(ab + Wb.a1) - ln4
        b_g = consts.tile([1, 1], f32)
        nc.vector.tensor_scalar(b_g[:], sb_ps, ab_sb[:], 0.8, A.add, A.mult)
        nc.vector.tensor_scalar(b_g[:], b_g[:], -LN4, None, A.add)
        # fsrc row [1, 2048] (no bias folded; it lives in b_g)
        fs_ps = ps_big.tile([1, N], f32, tag="big")
        for ch in range(4):
            for c in range(2):
                nc.tensor.matmul(
                    fs_ps[:, ch * 512:(ch + 1) * 512],
                    q16[:, c:c + 1],
                    xt_sb[:, c * N + ch * 512: c * N + ch * 512 + 512],
                    start=(c == 0), stop=(c == 1),
                )
        g_row = consts.tile([1, N], f16)
        nc.scalar.activation(g_row[:], fs_ps[:], AF.Exp, bias=b_g[:], scale=0.8)

        # -------- fdst columns straight from x (32 tiny matmuls) + wb.a2 row
        fd_ps = sm_ps[:, 8:24]
        for u in range(TB):
            for c in range(2):
                nc.tensor.matmul(
                    fd_ps[:, u:u + 1],
                    xt_sb[:, c * N + u * 128: c * N + u * 128 + 128],
                    q16[:, 2 + c:3 + c], start=(c == 0), stop=False,
                )
            nc.tensor.matmul(
                fd_ps[:, u:u + 1], ones_row[:], sb2_16[:],
                start=False, stop=True,
            )

        # -------- Gb[p, i] = g_i broadcast (outer product, chunked evac)
        ones_row = consts.tile([1, 128], f16)
        nc.vector.memset(ones_row[:], 1.0)
        gb_ps = ps_big.tile([128, N], f32, tag="big")
        gb = consts.tile([128, N], f16)

        def fs_chunk(ch):
            fs_ps = ps_gn.tile([1, 512], f32, tag="gn", name="fs_ps")
            for c in range(2):
                nc.tensor.matmul(
                    fs_ps[:],
                    q16[:, c:c + 1],
                    xt_sb[:, c * N + ch * 512: c * N + ch * 512 + 512],
                    start=(c == 0), stop=(c == 1),
                )
            nc.scalar.activation(
                g_row[:, ch * 512:(ch + 1) * 512], fs_ps[:],
                AF.Exp, bias=b_g[:], scale=0.8,
            )

        def gb_chunk(ch):
            gb_ps = ps_gn.tile([128, 512], f32, tag="gn", name="gb_ps")
            nc.tensor.matmul(
                gb_ps[:], ones_row[:],
                g_row[:, ch * 512:(ch + 1) * 512], start=True, stop=True,
            )
            nc.scalar.activation(
                gb[:, ch * 512:(ch + 1) * 512], gb_ps[:], AF.Copy,
            )

        fs_chunk(0)
        fs_chunk(1)
        gb_chunk(0)
        fs_chunk(2)
        gb_chunk(1)
        fs_chunk(3)
        gb_chunk(2)
        gb_chunk(3)

        # -------- nodes^T = W @ x^T + b -> f16
        nT_ps = ps_big.tile([128, N], f32, tag="big")
        for nch in range(4):
            for c in range(2):
                nc.tensor.matmul(
                    nT_ps[:, nch * 512:(nch + 1) * 512],
                    wt_sb[:, c * F:(c + 1) * F],
                    xt_sb[:, c * N + nch * 512: c * N + nch * 512 + 512],
                    start=(c == 0), stop=(c == 1),
                )
        nT16 = consts.tile([128, N], f16)
        nc.scalar.activation(nT16[:], nT_ps[:], AF.Identity, bias=wb_col[:])

        bln4 = consts.tile([128, 1], f32)
        nc.vector.memset(bln4[:], -LN4)
        c_cols = consts.tile([128, TB], f32)
        # -------- Gb[p, i] = g_i broadcast, chunk 0 first (gates block 0)
        gb = consts.tile([128, N], f16)
        gb_ps0 = ps_gn.tile([128, 512], f32, tag="gn", name="gb_ps0")
        nc.tensor.matmul(gb_ps0[:], ones_row[:], g_row[:, 0:512],
                         start=True, stop=True)
        nc.scalar.activation(gb[:, 0:512], gb_ps0[:], AF.Copy)
        nc.scalar.activation(c_cols[:], fd_ps, AF.Exp, scale=-0.8, bias=bln4[:])
        wv_cols = consts.tile([128, TB], f32)
        nc.scalar.activation(wv_cols[:], fd_ps, AF.Exp, scale=1.0)
        # Cbig[p, (u, i)] = c_{u*128+p}
        cbig = consts.tile([128, N], f16)
        ones128 = consts.tile([128, 128], f16)
        nc.vector.memset(ones128[:], 1.0)
        for u in range(TB):
            nc.vector.tensor_scalar(
                cbig[:, u * 128:(u + 1) * 128], ones128[:],
                c_cols[:, u:u + 1], None, A.mult,
            )

        # -------- nE[p, (u, e)] f16: cols 0..127 = v_j*nodes[j,:], col 128 = v_j
        nE_sb = consts.tile([128, TB * 129], f16)
        nE_v = nE_sb[:].rearrange("p (t e) -> p t e", e=129)
        for g4 in range(4):
            nE_ps = ps_gn.tile([128, 512], f16, tag="gn", name="nE_ps")
            for k in range(4):
                t = g4 * 4 + k
                nc.tensor.transpose(
                    nE_ps[:, k * 128:(k + 1) * 128],
                    nT16[:, t * 128:(t + 1) * 128],
                    idf16[:],
                )
            for k in range(4):
                u = g4 * 4 + k
                nc.scalar.activation(
                    nE_v[:, u, 0:128], nE_ps[:, k * 128:(k + 1) * 128],
                    AF.Copy, scale=v_cols[:, u:u + 1],
                )
        nc.vector.tensor_copy(
            nE_v[:, :, 128:129], wv_cols[:].rearrange("p (t o) -> p t o", o=1)
        )

        # -------- main loop: block pairs share one mask DMA --------
        RA, RD = RELU_ACT, RELU_DVE
        for tp in [tp for _ in range(reps) for tp in range(TB // 2)]:
            t1d = t1p.tile([128, 2 * N], f16, tag="t1")
            for h in range(2):
                t = 2 * tp + h
                for u in range(TB):
                    nc.vector.tensor_scalar(
                        t1d[:, h * N + u * 128: h * N + (u + 1) * 128],
                        gb[:, t * 128:(t + 1) * 128],
                        w_cols[:, u:u + 1], QUART, A.mult, A.max,
                    )
            nc.gpsimd.dma_start(
                t1d[:].rearrange("p (h m) -> p h m", h=2),
                adjm_d[2 * tp * 128:(2 * tp + 2) * 128, :].rearrange(
                    "(h p) m -> p h m", p=128),
                accum_op=A.add,
            )
            for h in range(2):
                t = 2 * tp + h
                o = h * N
                et = etp.tile([128, N], f16, tag="et")
                nc.scalar.activation(
                    et[:, 0:RA], t1d[:, o:o + RA], AF.Relu)
                nc.vector.tensor_scalar(
                    et[:, RA:RA + RD], t1d[:, o + RA:o + RA + RD],
                    0.0, None, A.max)
                nc.gpsimd.tensor_scalar(
                    et[:, RA + RD:], t1d[:, o + RA + RD:o + N],
                    0.0, None, A.max)

                out_ps = ps_out.tile([128, 129], f32, tag="out")
                for u in range(TB):
                    nc.tensor.matmul(
                        out_ps[:],
                        et[:, u * 128:(u + 1) * 128],
                        nE_v[:, u, :],
                        start=(u == 0), stop=(u == TB - 1),
                    )
                rcp = outp.tile([128, 1], f32, tag="rcp")
                nc.vector.reciprocal(rcp[:], out_ps[:, 128:129])
                osb = outp.tile([128, F], f16, tag="osb")
                nc.scalar.activation(osb[:], out_ps[:, 0:F], AF.Copy,
                                     scale=rcp[:])
                nc.sync.dma_start(out_d[t * 128:(t + 1) * 128, :], osb[:])

    nc.compile()
    return nc


def make_in_maps(inputs, adjs, W_w, W_b, a_w, a_b):
    xT = np.ascontiguousarray(
        np.asarray(inputs, dtype=np.float32).transpose(0, 2, 1)
    ).astype(np.float16)
    a = np.asarray(adjs, dtype=np.int8)
    # adjm[t*128+p, u*128+i] = -120*(1 - adj[b, t*128+i, u*128+p])
    a4 = a.reshape(BS, TB, 128, TB, 128)  # [b, t, i, u, p]
    adjm = ((a4.transpose(0, 1, 4, 3, 2).astype(np.int16) - 1) * 120).astype(
        np.int8
    ).reshape(BS, N, N)
    ww = np.ascontiguousarray(np.asarray(W_w, dtype=np.float32)).astype(np.float16)
    wwT = np.ascontiguousarray(ww.T)
    wb = np.ascontiguousarray(W_b, dtype=np.float32).reshape(F, 1)
    aw = np.asarray(a_w, dtype=np.float32)
    a12 = np.stack([aw[0, :F], aw[0, F:]], axis=1).astype(np.float16)
    ab = np.asarray(a_b, dtype=np.float32).reshape(1, 1).copy()
    idf16 = np.eye(128, dtype=np.float16)
    return [
        {
            "xT": xT[b],
            "adjm": adjm[b],
            "WwT": wwT,
            "Ww": ww,
            "Wb": wb,
            "Wb16": wb.astype(np.float16),
            "a12": a12,
            "ab": ab,
            "idf16": idf16,
        }
        for b in range(BS)
    ]


def kernel(inputs, adjs, W_w, W_b, a_w, a_b):
    from concourse.bass_utils import run_bass_kernel_spmd

    if "nc" not in _cache:
        _cache["nc"] = _build()
    nc = _cache["nc"]

    in_maps = make_in_maps(inputs, adjs, W_w, W_b, a_w, a_b)
    try:
        res = run_bass_kernel_spmd(nc, in_maps, core_ids=list(range(BS)))
    except Exception:
        res = run_bass_kernel_spmd(nc, in_maps, core_ids=list(range(BS)))
    out = np.stack([res.results[b]["out"] for b in range(BS)], axis=0)
    return out.astype(np.float32)
